# revision 35
# baseline (speedup 1.0000x reference)
"""Causal linear attention (fast_transformers style) on 8 Trainium2 cores.

query (8, 512, 64, 128) f32; Wq/Wk/Wv (128,128); bq/bk/bv (128,).
Data-parallel over batch B=8 -> one batch element per NeuronCore.

Per core (Bass/Tile kernel):
  x^T per node sequence arrives feat-major (F=128 partitions, L=512).
  qT = Wq.T @ xT, kT = Wk.T @ xT  (feat-major, PSUM)
  phi(x) = elu(x)+1 = exp(min(x,0)) + relu(x)   (ACT Exp/Relu + DVE)
  chunked causal attention, chunk C=128:
    AT_h = phi_k_h @ phi_q_h^T        (s,t) per head, via K=64 strips with
                                      per-partition head-select masks on q
    ATm = AT * triu(s<=t)             (DVE PSUM->SBUF move, bf16)
    intra_h = ATm_h.T @ [v_h|1]       -> out PSUM (t, 17h+f), start group
    inter   = qT_c.T @ S_bd           -> accumulate (exclusive KV state)
    S      += kt_c.T.T @ [v|1]        (kt_c = PE-transposed phi_k chunk)
    out     = num * recip(den + eps)  -> DMA to DRAM t-major
"""

import numpy as np
from concurrent.futures import ThreadPoolExecutor

HEADS = 8
E = 16
L = 512
NSEQ = 64
F = 128
C = 128
NCHUNK = L // C
EPS = 1e-6
NCORES = 8
VX = HEADS * (E + 1)  # 136: per-head [v (16) | ones] column blocks


def build_nc(nseq=NSEQ, vbias=True):
    import concourse.bass as bass
    import concourse.bacc as bacc
    import concourse.tile as tile
    from concourse import mybir

    dt = mybir.dt
    nc = bacc.Bacc("TRN2", target_bir_lowering=False, debug=False)

    xT = nc.dram_tensor("xT", [nseq, F, L], dt.bfloat16, kind="ExternalInput")
    wq = nc.dram_tensor("wq", [F, F], dt.bfloat16, kind="ExternalInput")
    wk = nc.dram_tensor("wk", [F, F], dt.bfloat16, kind="ExternalInput")
    wv = nc.dram_tensor("wv", [F, F], dt.bfloat16, kind="ExternalInput")
    bq = nc.dram_tensor("bq", [F, 1], dt.float32, kind="ExternalInput")
    bk = nc.dram_tensor("bk", [F, 1], dt.float32, kind="ExternalInput")
    nbq = nc.dram_tensor("nbq", [F, 1], dt.float32, kind="ExternalInput")
    nbk = nc.dram_tensor("nbk", [F, 1], dt.float32, kind="ExternalInput")
    bvb = nc.dram_tensor("bvb", [F, F], dt.bfloat16, kind="ExternalInput")
    triu = nc.dram_tensor("triu", [C, HEADS * C], dt.bfloat16, kind="ExternalInput")
    bdiag = nc.dram_tensor("bdiag", [F, VX], dt.bfloat16, kind="ExternalInput")
    qmask = nc.dram_tensor("qmask", [F, 4], dt.float32, kind="ExternalInput")
    ident = nc.dram_tensor("ident", [F, F], dt.bfloat16, kind="ExternalInput")
    y = nc.dram_tensor("y", [L, nseq, F], dt.bfloat16, kind="ExternalOutput")

    with tile.TileContext(nc) as tc:
        with (
            tc.tile_pool(name="consts", bufs=1) as consts,
            tc.tile_pool(name="xp", bufs=3) as xp,
            tc.tile_pool(name="phip", bufs=3) as phip,
            tc.tile_pool(name="tmp", bufs=3) as tmp,
            tc.tile_pool(name="ktp", bufs=2) as ktp,
            tc.tile_pool(name="atm", bufs=3) as atmp,
            tc.tile_pool(name="outp", bufs=3) as outp,
            tc.tile_pool(name="ps_q", bufs=1, space="PSUM") as ps_q,
            tc.tile_pool(name="ps_k", bufs=1, space="PSUM") as ps_k,
            tc.tile_pool(name="ps_a", bufs=1, space="PSUM") as ps_a,
            tc.tile_pool(name="ps_o", bufs=2, space="PSUM") as ps_o,
            tc.tile_pool(name="ps_s", bufs=1, space="PSUM") as ps_s,
            tc.tile_pool(name="ps_v", bufs=1, space="PSUM") as ps_v,
        ):
            wq_t = consts.tile([F, F], dt.bfloat16)
            nc.sync.dma_start(wq_t[:], wq[:])
            wk_t = consts.tile([F, F], dt.bfloat16)
            nc.sync.dma_start(wk_t[:], wk[:])
            wv_t = consts.tile([F, F], dt.bfloat16)
            nc.sync.dma_start(wv_t[:], wv[:])
            bq_t = consts.tile([F, 1], dt.float32)
            nc.sync.dma_start(bq_t[:], bq[:])
            bk_t = consts.tile([F, 1], dt.float32)
            nc.sync.dma_start(bk_t[:], bk[:])
            bvb_t = consts.tile([F, F], dt.bfloat16)
            nc.sync.dma_start(bvb_t[:], bvb[:])
            triu_t = consts.tile([C, HEADS * C], dt.bfloat16)
            nc.sync.dma_start(triu_t[:], triu[:])
            bdiag_t = consts.tile([F, VX], dt.bfloat16)
            nc.sync.dma_start(bdiag_t[:], bdiag[:])
            qm_t = consts.tile([F, 4], dt.float32)
            nc.sync.dma_start(qm_t[:], qmask[:])
            id_t = consts.tile([F, F], dt.bfloat16)
            nc.sync.dma_start(id_t[:], ident[:])

            # persistent manually double-buffered vex tiles: the ones
            # columns survive across sequences, memset once
            vex_bufs = []
            for p in range(2):
                row = []
                for c in range(NCHUNK):
                    vx = consts.tile([C, VX], dt.bfloat16, name=f"vexb{p}_{c}")
                    nc.vector.memset(vx[:, E::E + 1], 1.0)
                    row.append(vx)
                vex_bufs.append(row)

            for n in range(nseq):
                x_t = xp.tile([F, L], dt.bfloat16)
                nc.sync.dma_start(x_t[:], xT[n])

                # ---- projections (feat-major) ----
                q_ps = ps_q.tile([F, L], dt.float32)
                nc.tensor.matmul(q_ps[:], wq_t[:], x_t[:], start=True, stop=True)
                k_ps = ps_k.tile([F, L], dt.float32)
                nc.tensor.matmul(k_ps[:], wk_t[:], x_t[:], start=True, stop=True)

                # ---- phi = elu(x+b)+1 = min(exp(x+b), 1) + relu(x+b) ----
                # (exp(xb) >= xb+1 always; values bounded ~e^6, safe in bf16)
                def phi(ps, bias_t, name):
                    e = tmp.tile([F, L], dt.bfloat16, tag="e")
                    nc.scalar.activation(
                        e[:], ps[:], mybir.ActivationFunctionType.Exp,
                        bias=bias_t[:], scale=1.0)
                    r = tmp.tile([F, L], dt.bfloat16, tag="r")
                    nc.scalar.activation(
                        r[:], ps[:], mybir.ActivationFunctionType.Relu,
                        bias=bias_t[:], scale=1.0)
                    ph = phip.tile([F, L], dt.bfloat16, tag=name)
                    nc.vector.scalar_tensor_tensor(
                        ph[:], e[:], 1.0, r[:],
                        op0=mybir.AluOpType.min, op1=mybir.AluOpType.add)
                    return ph

                phq = phi(q_ps, bq_t, "phq")
                phk = phi(k_ps, bk_t, "phk")

                # head-select masked copies of phi_q, interleaved per chunk
                # [qm0_c | qm1_c | qm2_c | qm3_c]; split DVE / ACT
                qtile = phip.tile([F, 4 * L], dt.bfloat16, tag="qtile")
                q4 = qtile[:].rearrange("p (c j x) -> p c j x", c=NCHUNK, j=4)
                src3 = phq[:].rearrange("p (c x) -> p c x", c=NCHUNK)
                for j in range(4):
                    if j < 2:
                        nc.vector.tensor_scalar(
                            q4[:, :, j, :], src3, qm_t[:, j:j + 1], None,
                            op0=mybir.AluOpType.mult)
                    else:
                        nc.scalar.mul(q4[:, :, j, :], src3, qm_t[:, j:j + 1])

                # ---- per chunk: v, kt (t-major), attention ----
                vex = vex_bufs[n % 2]
                kts = []
                for c in range(NCHUNK):
                    cs = slice(c * C, (c + 1) * C)
                    v_ps = ps_v.tile([C, F], dt.float32, tag="pv")
                    nc.tensor.matmul(v_ps[:], x_t[:, cs], wv_t[:],
                                     start=True, stop=True)
                    vx = vex[c]
                    # vex[:, 17h+f] = v_ps[:, 16h+f] + bv  (strided dest)
                    dst = vx[:].rearrange("p (h x) -> p h x", h=HEADS)[:, :, 0:E]
                    src = v_ps[:].rearrange("p (h x) -> p h x", h=HEADS)
                    if vbias:
                        bsr = bvb_t[:].rearrange("p (h x) -> p h x", h=HEADS)
                        nc.vector.tensor_add(dst, src, bsr)
                    else:
                        # zero bias: plain move, off the vector engine
                        nc.scalar.activation(
                            dst, src, mybir.ActivationFunctionType.Copy)

                    # t-major phi(k) via PE transpose (only chunks 0..2 are
                    # ever read by the state update)
                    if c < NCHUNK - 1:
                        kt_ps = ps_v.tile([C, F], dt.bfloat16, tag="pv",
                                          name=f"kt_ps{c}")
                        nc.tensor.transpose(kt_ps[:], phk[:, cs], id_t[:])
                        kt = ktp.tile([C, F], dt.bfloat16, tag=f"kt{c}")
                        nc.scalar.activation(
                            kt[:], kt_ps[:], mybir.ActivationFunctionType.Copy)
                        kts.append(kt)

                S_ps = ps_s.tile([F, VX], dt.float32)

                for c in range(NCHUNK):
                    cs = slice(c * C, (c + 1) * C)
                    # scores AT_h (s,t): K=64 halves x double-width masked-q
                    # rhs -> 4 matmuls, each computing a head pair
                    a_ps = ps_a.tile([C, HEADS * C], dt.float32)
                    for half in range(2):
                        rows = slice(64 * half, 64 * (half + 1))
                        for pb in (0, 2):
                            h0 = 4 * half + pb
                            nc.tensor.matmul(
                                a_ps[:, h0 * C:(h0 + 2) * C],
                                phk[rows, cs],
                                qtile[rows, c * 4 * C + pb * C:
                                      c * 4 * C + (pb + 2) * C],
                                start=True, stop=True)
                    # masked move PSUM->SBUF split: DVE 5 heads direct,
                    # ACT copies 3 heads, DVE masks those from SBUF (2x bf16)
                    atm = atmp.tile([C, HEADS * C], dt.bfloat16)
                    nc.vector.tensor_mul(
                        atm[:, 0:5 * C], a_ps[:, 0:5 * C], triu_t[:, 0:5 * C])
                    a_sb = tmp.tile([C, 3 * C], dt.bfloat16, tag="a_sb")
                    nc.scalar.activation(
                        a_sb[:], a_ps[:, 5 * C:], mybir.ActivationFunctionType.Copy)
                    nc.vector.tensor_mul(
                        atm[:, 5 * C:], a_sb[:], triu_t[:, 5 * C:])

                    # output accumulator (t, [num16|den1] x 8)
                    o_ps = ps_o.tile([C, VX], dt.float32)
                    for h in range(HEADS):
                        nc.tensor.matmul(
                            o_ps[:, h * (E + 1):(h + 1) * (E + 1)],
                            atm[:, h * C:(h + 1) * C], vex[c][:, h * (E + 1):(h + 1) * (E + 1)],
                            start=(h == 0), stop=(c == 0 and h == HEADS - 1))
                    if c > 0:
                        sbd = tmp.tile([F, VX], dt.bfloat16, tag="sbd")
                        nc.vector.tensor_mul(sbd[:], S_ps[:], bdiag_t[:])
                        nc.tensor.matmul(o_ps[:], phq[:, cs], sbd[:],
                                         start=False, stop=True)
                    # state += kv (after exclusive copy above); the last
                    # chunk's update is never read -> skip it
                    if c < NCHUNK - 1:
                        nc.tensor.matmul(S_ps[:], kts[c][:], vex[c][:],
                                         start=(c == 0), stop=True,
                                         skip_group_check=(c > 0))

                    # normalize: out = num / (den + eps)
                    o3 = o_ps[:].rearrange("p (h x) -> p h x", h=HEADS)
                    den = tmp.tile([C, HEADS], dt.float32, tag="den")
                    nc.vector.tensor_scalar(
                        den[:], o3[:, :, E], EPS, None, op0=mybir.AluOpType.add)
                    rin = tmp.tile([C, HEADS], dt.float32, tag="rin")
                    nc.vector.reciprocal(rin[:], den[:])
                    o_sb = outp.tile([C, F], dt.bfloat16)
                    o_sb3 = o_sb[:].rearrange("p (h x) -> p h x", h=HEADS)
                    rin3 = rin[:][:, :, None].broadcast_to([C, HEADS, E])
                    nc.vector.tensor_mul(o_sb3, o3[:, :, 0:E], rin3)
                    nc.sync.dma_start(y[cs, n, :], o_sb[:])
    nc.finalize()
    return nc


_BF16 = None


def _bf16():
    global _BF16
    if _BF16 is None:
        import ml_dtypes
        _BF16 = ml_dtypes.bfloat16
    return _BF16


def _consts(Wq, bq, Wk, bk, Wv, bv):
    bf16 = _bf16()
    tri = np.triu(np.ones((C, C), np.float32))  # AT[s,t] keep s<=t
    triu_rep = np.tile(tri, (1, HEADS)).astype(bf16)
    bd = np.zeros((F, VX), np.float32)
    for h in range(HEADS):
        bd[h * E:(h + 1) * E, h * (E + 1):(h + 1) * (E + 1)] = 1.0
    qm = np.zeros((F, 4), np.float32)
    for p in range(F):
        qm[p, (p % 64) // E] = 1.0
    return {
        "wq": np.ascontiguousarray(Wq.astype(bf16)),
        "wk": np.ascontiguousarray(Wk.astype(bf16)),
        "wv": np.ascontiguousarray(Wv.astype(bf16)),
        "bq": np.ascontiguousarray(bq.reshape(F, 1).astype(np.float32)),
        "bk": np.ascontiguousarray(bk.reshape(F, 1).astype(np.float32)),
        "nbq": np.ascontiguousarray(-bq.reshape(F, 1).astype(np.float32)),
        "nbk": np.ascontiguousarray(-bk.reshape(F, 1).astype(np.float32)),
        "bvb": np.ascontiguousarray(
            np.broadcast_to(bv.astype(np.float32), (F, F)).astype(bf16)),
        "triu": triu_rep,
        "bdiag": bd.astype(bf16),
        "qmask": qm,
        "ident": np.eye(F, dtype=bf16),
    }


_NC = {}
TRACE = False
TRACE_DIR = None
LAST_RESULT = None


def kernel(query, Wq, bq, Wk, bk, Wv, bv):
    global LAST_RESULT
    from concourse import bass_utils

    bf16 = _bf16()
    consts = _consts(Wq, bq, Wk, bk, Wv, bv)

    def prep(c):
        # (L, N, F) -> (N, F, L) bf16, feat-major per sequence
        return np.ascontiguousarray(
            np.asarray(query[c]).transpose(1, 2, 0)).astype(bf16)

    with ThreadPoolExecutor(NCORES) as ex:
        xts = list(ex.map(prep, range(NCORES)))

    in_maps = [dict(consts, xT=xts[c]) for c in range(NCORES)]

    vb = bool(np.any(np.asarray(bv) != 0))
    if vb not in _NC:
        _NC[vb] = build_nc(vbias=vb)
    kwargs = {}
    if TRACE:
        kwargs = dict(trace=True, tmpdir=TRACE_DIR)
    res = bass_utils.run_bass_kernel_spmd(
        _NC[vb], in_maps, core_ids=list(range(NCORES)), **kwargs)
    LAST_RESULT = res
    out = np.empty((NCORES, L, NSEQ, F), np.float32)
    for c in range(NCORES):
        out[c] = res.results[c]["y"].astype(np.float32)
    return out
